# revision 8
# baseline (speedup 1.0000x reference)
"""Trainium2 Bass kernel for nn_GATPredictor: 2-layer LSTM + 2-layer GAT + head.

Sharding: data-parallel over batch B=8 across the 8 NeuronCores (one batch
element per core); adjacency and weights replicated. Per core:
  - LSTM over T=12 steps, hidden 64, feat-major layout (feat on partitions),
    f32r matmuls, sigmoid/tanh on ScalarE with fused per-partition bias.
  - GAT: E_T[j,i] = exp(lrelu(s_i + d_j + maskneg_ji)) built tilewise
    (partition=j, free=i); attention contraction as f32r matmuls
    lhsT=hp_aug(j,cols), rhs=E_T(j,i) accumulating (16,i) per head plus a
    separate Z (softmax denominator) row; normalize+relu into padded
    per-head-pair tiles (heads at 32-aligned row offsets).
"""
import numpy as np
import ml_dtypes

import concourse.bacc as bacc
import concourse.tile as tile
import concourse.mybir as mybir
from concourse.bass_utils import run_bass_kernel_spmd

F32 = mybir.dt.float32
F32R = mybir.dt.float32r
BF16 = mybir.dt.bfloat16
AF = mybir.ActivationFunctionType
ALU = mybir.AluOpType

B, T, N = 8, 12, 1500
HID, HEADS, DH, OUT = 64, 4, 16, 3
G4 = 4 * HID  # 256 gates
NT = (N + 127) // 128           # 12 node tiles (last has 92 rows)
CHUNKS = [(0, 512), (512, 512), (1024, 476)]  # matmul N-chunks (even sizes)
MASKNEG = -300.0

E_BF16 = False  # E/hp in bf16 (faster DVE/matmul) vs f32r (more accurate)


def _ntcnt(nt):
    return min(128, N - nt * 128)


def _build():
    E_DT = BF16 if E_BF16 else F32
    E_MM = BF16 if E_BF16 else F32R

    nc = bacc.Bacc("TRN2", target_bir_lowering=False, debug=False, num_devices=8)

    xc = nc.dram_tensor("xc", [T, N], F32, kind="ExternalInput")
    maskneg = nc.dram_tensor("maskneg", [N, N], BF16, kind="ExternalInput")
    wcomb0 = nc.dram_tensor("wcomb0", [HID, G4], F32, kind="ExternalInput")
    wih0 = nc.dram_tensor("wih0", [1, G4], F32, kind="ExternalInput")
    wcomb1 = nc.dram_tensor("wcomb1", [2 * HID, G4], F32, kind="ExternalInput")
    bias0 = nc.dram_tensor("bias0", [G4, 1], F32, kind="ExternalInput")
    bias1 = nc.dram_tensor("bias1", [G4, 1], F32, kind="ExternalInput")
    # layer-0 GAT weights: plain layout (input is contiguous 64-feat slab)
    waug0 = nc.dram_tensor("waug0", [HID, 66], F32, kind="ExternalInput")
    acomb0 = nc.dram_tensor("acomb0", [HID, 8], F32, kind="ExternalInput")
    # layer-1 GAT + head weights: padded layout (input rows 32*h+d, d<16)
    waug1p = nc.dram_tensor("waug1p", [128, 66], F32, kind="ExternalInput")
    acomb1p = nc.dram_tensor("acomb1p", [128, 8], F32, kind="ExternalInput")
    woutp = nc.dram_tensor("woutp", [128, 4], F32, kind="ExternalInput")
    wob = nc.dram_tensor("wob", [1, 4], F32, kind="ExternalInput")
    out_d = nc.dram_tensor("out", [OUT, N], F32, kind="ExternalOutput")

    ident2_d = nc.inline_tensor(np.eye(2, dtype=np.float32), name="ident2")
    ones66_np = np.zeros((1, 66), np.float32)
    ones66_np[0, 64] = 1.0
    ones66_d = nc.inline_tensor(ones66_np, name="ones66")

    with tile.TileContext(nc) as tc:
        with tc.tile_pool(name="perm", bufs=1) as perm:
            stg_pool = tc.tile_pool(name="stage", bufs=1)
            stg = stg_pool.__enter__()

            # ---- persistent tiles ----
            def load_w(dram, p, q, name, row0=None):
                f = stg.tile([p, q], F32, name=name + "_f", tag=name + "_f")
                if row0 is None:
                    nc.sync.dma_start(f[:], dram[:])
                else:
                    nc.sync.dma_start(f[:], dram[row0:row0 + p, :])
                r = perm.tile([p, q], F32R, name=name, tag=name)
                nc.vector.tensor_copy(r[:], f[:])
                return r

            wc0 = load_w(wcomb0, HID, G4, "wc0")
            wi0 = load_w(wih0, 1, G4, "wi0")
            wc1 = load_w(wcomb1, 2 * HID, G4, "wc1")
            wg0 = load_w(waug0, HID, 66, "wg0")
            wg1 = [load_w(waug1p, 64, 66, f"wg1_{i}", row0=64 * i)
                   for i in range(2)]
            ac0 = load_w(acomb0, HID, 8, "ac0")
            ac1 = [load_w(acomb1p, 64, 8, f"ac1_{i}", row0=64 * i)
                   for i in range(2)]
            wop = [load_w(woutp, 64, 4, f"wop{i}", row0=64 * i)
                   for i in range(2)]
            wo_one = load_w(wob, 1, 4, "wo_one")
            wg_one = load_w(ones66_d, 1, 66, "wg_one")

            bA, bB = [], []
            for li, bd in enumerate((bias0, bias1)):
                ba = perm.tile([128, 1], F32, name=f"bA{li}", tag=f"bA{li}")
                bb = perm.tile([128, 1], F32, name=f"bB{li}", tag=f"bB{li}")
                nc.sync.dma_start(ba[:], bd[0:128, :])
                nc.sync.dma_start(bb[:], bd[128:256, :])
                bA.append(ba)
                bB.append(bb)
            id2 = perm.tile([2, 2], F32)
            nc.sync.dma_start(id2[:], ident2_d[:])

            # ones row (f32r) for ones/bias matmul terms
            ones_f = stg.tile([1, N], F32)
            nc.gpsimd.memset(ones_f[:], 1.0)
            ones_r = perm.tile([1, N], F32R)
            nc.gpsimd.tensor_copy(ones_r[:], ones_f[:])

            # mask tiles (transposed adjacency, additive -300), resident
            mk = []
            for nt in range(NT):
                cnt = _ntcnt(nt)
                m = perm.tile([128, N], BF16, tag=f"mk{nt}", name=f"mk{nt}")
                nc.sync.dma_start(m[0:cnt, :], maskneg[nt * 128:nt * 128 + cnt, :])
                mk.append(m)

            # LSTM state [h0(0:64); h1(64:128)], f32r
            state1 = perm.tile([128, N], F32R)
            ctiles = [perm.tile([128, N], F32, tag=f"c{j}", name=f"c{j}")
                      for j in range(2)]

            # GAT outputs: per layer, two (64,N) tiles; head h at tile h//2,
            # rows 32*(h%2):32*(h%2)+16; other rows stay zero.
            zf = stg.tile([64, N], F32)
            nc.gpsimd.memset(zf[:], 0.0)
            gh = [[perm.tile([64, N], F32R, tag=f"gh{g}_{i}", name=f"gh{g}_{i}")
                   for i in range(2)] for g in range(2)]
            for g in range(2):
                for i in range(2):
                    nc.vector.tensor_copy(gh[g][i][:], zf[:])

            stg_pool.__exit__(None, None, None)

            # =================== LSTM ===================
            with tc.tile_pool(name="lwork", bufs=2) as lw, \
                 tc.tile_pool(name="lpsum", bufs=1, space="PSUM") as lp:
                for t in range(T):
                    xr_f = lw.tile([1, N], F32, tag="xrf", name="xrf")
                    nc.sync.dma_start(xr_f[:], xc[t:t + 1, :])
                    xr = lw.tile([1, N], F32R, tag="xrr", name="xrr")
                    nc.gpsimd.tensor_copy(xr[:], xr_f[:])
                    for l in range(2):
                        for ci, (s, w) in enumerate(CHUNKS):
                            gA = lp.tile([128, w], F32, tag=f"gA{ci}",
                                         name=f"gA{ci}")
                            gB = lp.tile([128, w], F32, tag=f"gB{ci}",
                                         name=f"gB{ci}")
                            if l == 0:
                                if t == 0:
                                    nc.tensor.matmul(gA[:], wi0[:, 0:128],
                                                     xr[:, s:s + w],
                                                     start=True, stop=True)
                                    nc.tensor.matmul(gB[:], wi0[:, 128:256],
                                                     xr[:, s:s + w],
                                                     start=True, stop=True)
                                else:
                                    nc.tensor.matmul(gA[:], wc0[:, 0:128],
                                                     state1[0:64, s:s + w],
                                                     start=True, stop=False)
                                    nc.tensor.matmul(gA[:], wi0[:, 0:128],
                                                     xr[:, s:s + w],
                                                     start=False, stop=True)
                                    nc.tensor.matmul(gB[:], wc0[:, 128:256],
                                                     state1[0:64, s:s + w],
                                                     start=True, stop=False)
                                    nc.tensor.matmul(gB[:], wi0[:, 128:256],
                                                     xr[:, s:s + w],
                                                     start=False, stop=True)
                            else:
                                if t == 0:
                                    nc.tensor.matmul(gA[:], wc1[0:64, 0:128],
                                                     state1[0:64, s:s + w],
                                                     start=True, stop=True)
                                    nc.tensor.matmul(gB[:], wc1[0:64, 128:256],
                                                     state1[0:64, s:s + w],
                                                     start=True, stop=True)
                                else:
                                    nc.tensor.matmul(gA[:], wc1[:, 0:128],
                                                     state1[:, s:s + w],
                                                     start=True, stop=True)
                                    nc.tensor.matmul(gB[:], wc1[:, 128:256],
                                                     state1[:, s:s + w],
                                                     start=True, stop=True)
                            # activations: gA=[i;f] sigmoid, gB=[g;o]
                            sA = lw.tile([128, w], F32, tag=f"sA{ci}",
                                         name=f"sA{ci}")
                            nc.scalar.activation(sA[:], gA[:], AF.Sigmoid,
                                                 bias=bA[l][:])
                            tg = lw.tile([64, w], F32, tag=f"tg{ci}",
                                         name=f"tg{ci}")
                            nc.scalar.activation(tg[:], gB[0:64, :], AF.Tanh,
                                                 bias=bB[l][0:64, :])
                            so = lw.tile([64, w], F32, tag=f"so{ci}",
                                         name=f"so{ci}")
                            nc.scalar.activation(so[:], gB[64:128, :], AF.Sigmoid,
                                                 bias=bB[l][64:128, :])
                            Cl = ctiles[l]
                            if t == 0:
                                nc.vector.tensor_mul(Cl[64:128, s:s + w],
                                                     sA[0:64, :], tg[:])
                            else:
                                M = lw.tile([64, 2 * w], F32, tag=f"M{ci}",
                                            name=f"M{ci}")
                                nc.vector.tensor_mul(M[:, 0:w], sA[0:64, :],
                                                     tg[:])
                                nc.gpsimd.tensor_mul(M[:, w:2 * w],
                                                     sA[64:128, :],
                                                     Cl[64:128, s:s + w])
                                nc.vector.tensor_add(Cl[64:128, s:s + w],
                                                     M[:, 0:w], M[:, w:2 * w])
                            tc_sb = lw.tile([64, w], F32, tag=f"tc{ci}",
                                            name=f"tc{ci}")
                            nc.scalar.activation(tc_sb[:], Cl[64:128, s:s + w],
                                                 AF.Tanh)
                            nc.vector.tensor_mul(
                                state1[64 * l:64 * l + 64, s:s + w],
                                so[:], tc_sb[:])

            # h1 (final top-layer hidden) to a base-0 feat-major tile
            h1t = perm.tile([64, N], F32R)
            nc.vector.tensor_copy(h1t[:], state1[64:128, :])

            # =================== GAT layers ===================
            for g in range(2):
                with tc.tile_pool(name=f"gw{g}", bufs=2) as gwp, \
                     tc.tile_pool(name=f"gp{g}", bufs=1, space="PSUM") as gp:
                    # hp_aug per node tile: (cnt, 66) cols 0:64 feats, 64=Z-ones
                    hpa = []
                    for nt in range(NT):
                        cnt = _ntcnt(nt)
                        ns = nt * 128
                        php = gp.tile([128, 66], F32, tag="php", name="php")
                        if g == 0:
                            nc.tensor.matmul(php[0:cnt, :],
                                             h1t[:, ns:ns + cnt],
                                             wg0[:],
                                             start=True, stop=False)
                        else:
                            for i in range(2):
                                nc.tensor.matmul(php[0:cnt, :],
                                                 gh[0][i][:, ns:ns + cnt],
                                                 wg1[i][:],
                                                 start=(i == 0), stop=False)
                        nc.tensor.matmul(php[0:cnt, :],
                                         ones_r[:, ns:ns + cnt],
                                         wg_one[:],
                                         start=False, stop=True)
                        hp = gwp.tile([128, 66], E_MM, tag=f"hp{nt}",
                                      name=f"hp{nt}", bufs=1)
                        nc.scalar.copy(hp[0:cnt, :], php[0:cnt, :])
                        hpa.append(hp)

                    for h in range(HEADS):
                        # e_src/e_dst rows for head h: (2, N) f32
                        esd = gwp.tile([2, N], F32, tag="esd", name="esd", bufs=1)
                        for ci, (s, w) in enumerate(CHUNKS):
                            pe = gp.tile([2, 512], F32, tag="pesd", name="pesd")
                            if g == 0:
                                nc.tensor.matmul(pe[:, 0:w],
                                                 ac0[:, 2 * h:2 * h + 2],
                                                 h1t[:, s:s + w],
                                                 start=True, stop=True)
                            else:
                                for i in range(2):
                                    nc.tensor.matmul(
                                        pe[:, 0:w],
                                        ac1[i][:, 2 * h:2 * h + 2],
                                        gh[0][i][:, s:s + w],
                                        start=(i == 0), stop=(i == 1))
                            nc.scalar.copy(esd[:, s:s + w], pe[:, 0:w])
                        # d columns via PE transpose: dcols[:, 2jt+1] = d
                        dcols = gwp.tile([128, 2 * NT], F32, tag="dcols",
                                         name="dcols", bufs=1)
                        for nt in range(NT):
                            cnt = _ntcnt(nt)
                            ptr = gp.tile([128, 2], F32, tag="pesd",
                                          name="ptr")
                            nc.tensor.transpose(ptr[0:cnt, :],
                                                esd[:, nt * 128:nt * 128 + cnt],
                                                id2[:])
                            nc.scalar.copy(dcols[0:cnt, 2 * nt:2 * nt + 2],
                                           ptr[0:cnt, :])
                        # s broadcast along partitions
                        if E_BF16:
                            esd_b = gwp.tile([1, N], BF16, tag="esdb",
                                             name="esdb")
                            nc.scalar.copy(esd_b[:], esd[0:1, :])
                            sb_src = esd_b[0:1, :]
                        else:
                            sb_src = esd[0:1, :]
                        sbc = gwp.tile([128, N], E_DT, tag="sbc", name="sbc", bufs=1)
                        nc.gpsimd.partition_broadcast(sbc[:], sb_src)

                        pho = gp.tile([16, N], F32, tag="pho", name="pho")
                        pz = gp.tile([2, N], F32, tag="pz", name="pz")
                        for nt in range(NT):
                            cnt = _ntcnt(nt)
                            tt = gwp.tile([128, N], E_DT, tag="tt", name="tt")
                            nc.vector.scalar_tensor_tensor(
                                tt[0:cnt, :], mk[nt][0:cnt, :],
                                dcols[0:cnt, 2 * nt + 1:2 * nt + 2],
                                sbc[0:cnt, :], ALU.add, ALU.add)
                            lr = gwp.tile([128, N], E_DT, tag="lr", name="lr")
                            nc.vector.scalar_tensor_tensor(
                                lr[0:cnt, :], tt[0:cnt, :], 0.2, tt[0:cnt, :],
                                ALU.mult, ALU.max)
                            em = gwp.tile([128, N], E_MM, tag="em", name="em")
                            nc.scalar.activation(em[0:cnt, :], lr[0:cnt, :],
                                                 AF.Exp)
                            for ci, (s, w) in enumerate(CHUNKS):
                                nc.tensor.matmul(
                                    pho[:, s:s + w],
                                    hpa[nt][0:cnt, 16 * h:16 * h + 16],
                                    em[0:cnt, s:s + w],
                                    start=(nt == 0), stop=(nt == NT - 1))
                                nc.tensor.matmul(
                                    pz[:, s:s + w],
                                    hpa[nt][0:cnt, 64:66],
                                    em[0:cnt, s:s + w],
                                    start=(nt == 0), stop=(nt == NT - 1))
                        # normalize + relu -> gh[g]
                        rz = gwp.tile([1, N], F32, tag="rz", name="rz", bufs=1)
                        nc.vector.reciprocal(rz[:], pz[0:1, :])
                        rzb = gwp.tile([16, N], F32, tag="rzb", name="rzb", bufs=1)
                        nc.gpsimd.partition_broadcast(rzb[:], rz[:])
                        rh = gwp.tile([16, N], F32, tag="rh", name="rh", bufs=1)
                        nc.scalar.activation(rh[:], pho[:], AF.Relu)
                        ro = 32 * (h % 2)
                        nc.vector.tensor_mul(gh[g][h // 2][ro:ro + 16, :],
                                             rh[:], rzb[:])

            # =================== output head ===================
            with tc.tile_pool(name="hw", bufs=1) as hw, \
                 tc.tile_pool(name="hp2", bufs=1, space="PSUM") as hp2:
                po = hp2.tile([4, N], F32, tag="po", name="po")
                for ci, (s, w) in enumerate(CHUNKS):
                    for i in range(2):
                        nc.tensor.matmul(po[:, s:s + w],
                                         wop[i][:],
                                         gh[1][i][:, s:s + w],
                                         start=(i == 0), stop=False)
                    nc.tensor.matmul(po[:, s:s + w], wo_one[:],
                                     ones_r[:, s:s + w],
                                     start=False, stop=True)
                osb = hw.tile([4, N], F32)
                nc.scalar.copy(osb[:], po[:])
                nc.sync.dma_start(out_d[:], osb[0:3, :])

    nc.compile()
    return nc


_NC = None


def _pad64(a):
    """(64, q) rows c=16h+d -> (128, q) rows 32h+d (d<16), zeros elsewhere."""
    out = np.zeros((128, a.shape[1]), np.float32)
    for h in range(HEADS):
        out[32 * h:32 * h + 16] = a[16 * h:16 * h + 16]
    return out


def _pack(inputs):
    f32 = lambda a: np.asarray(a, dtype=np.float32)
    adj = np.asarray(inputs["adj"])
    maskneg = np.where(adj.T > 0, 0.0, MASKNEG).astype(ml_dtypes.bfloat16)
    W0, W1 = f32(inputs["gat0_W"]), f32(inputs["gat1_W"])

    def gat_w(W, asrc, adst):
        wa = np.zeros((HID, 66), np.float32)
        wa[:, 0:HID] = W.T
        acb = np.zeros((HID, 8), np.float32)
        for h in range(HEADS):
            blk = W[16 * h:16 * h + 16, :]  # (16, 64)
            acb[:, 2 * h] = f32(asrc)[h] @ blk
            acb[:, 2 * h + 1] = f32(adst)[h] @ blk
        return wa, acb

    wa0, acb0 = gat_w(W0, inputs["gat0_asrc"], inputs["gat0_adst"])
    wa1, acb1 = gat_w(W1, inputs["gat1_asrc"], inputs["gat1_adst"])
    wo = np.zeros((HID, 4), np.float32)
    wo[:, 0:OUT] = f32(inputs["out_W"]).T
    wob = np.zeros((1, 4), np.float32)
    wob[0, 0:OUT] = f32(inputs["out_b"])
    shared = {
        "maskneg": maskneg,
        "wcomb0": f32(inputs["lstm_Whh0"]).T.copy(),
        "wih0": f32(inputs["lstm_Wih0"]).T.copy(),
        "wcomb1": np.concatenate([f32(inputs["lstm_Wih1"]).T,
                                  f32(inputs["lstm_Whh1"]).T], 0),
        "bias0": (f32(inputs["lstm_bih0"]) +
                  f32(inputs["lstm_bhh0"])).reshape(G4, 1),
        "bias1": (f32(inputs["lstm_bih1"]) +
                  f32(inputs["lstm_bhh1"])).reshape(G4, 1),
        "waug0": wa0,
        "acomb0": acb0,
        "waug1p": _pad64(wa1),
        "acomb1p": _pad64(acb1),
        "woutp": _pad64(wo),
        "wob": wob,
    }
    x = f32(inputs["x"])
    return [dict(shared, xc=x[b].copy()) for b in range(B)]


def kernel(**inputs) -> np.ndarray:
    global _NC
    in_maps = _pack(inputs)
    if _NC is None:
        _NC = _build()
    res = run_bass_kernel_spmd(_NC, in_maps, list(range(B)))
    return np.stack([res.results[b]["out"] for b in range(B)], 0)


# revision 9
# speedup vs baseline: 975.3119x; 975.3119x over previous
"""Trainium2 Bass kernel for nn_GATPredictor: 2-layer LSTM + 2-layer GAT + head.

Sharding: data-parallel over batch B=8 across the 8 NeuronCores (one batch
element per core); adjacency and weights replicated. Per core:
  - LSTM over T=12 steps, hidden 64, feat-major layout (feat on partitions),
    f32r matmuls, sigmoid/tanh on ScalarE with fused per-partition bias.
  - GAT: E_T[j,i] = exp(lrelu(s_i + d_j + maskneg_ji)) built tilewise
    (partition=j, free=i); attention contraction as f32r matmuls
    lhsT=hp_aug(j,cols), rhs=E_T(j,i) accumulating (16,i) per head plus a
    separate Z (softmax denominator) row; normalize+relu into padded
    per-head-pair tiles (heads at 32-aligned row offsets).
"""
import numpy as np
import ml_dtypes

import concourse.bacc as bacc
import concourse.tile as tile
import concourse.mybir as mybir
from concourse.bass_utils import run_bass_kernel_spmd

F32 = mybir.dt.float32
F32R = mybir.dt.float32r
BF16 = mybir.dt.bfloat16
AF = mybir.ActivationFunctionType
ALU = mybir.AluOpType

B, T, N = 8, 12, 1500
HID, HEADS, DH, OUT = 64, 4, 16, 3
G4 = 4 * HID  # 256 gates
NT = (N + 127) // 128           # 12 node tiles (last has 92 rows)
CHUNKS = [(0, 512), (512, 512), (1024, 476)]  # matmul N-chunks (even sizes)
MASKNEG = -300.0

E_BF16 = False  # E/hp in bf16 (faster DVE/matmul) vs f32r (more accurate)


def _ntcnt(nt):
    return min(128, N - nt * 128)


def _build(reps=1):
    E_DT = BF16 if E_BF16 else F32
    E_MM = BF16 if E_BF16 else F32R

    nc = bacc.Bacc("TRN2", target_bir_lowering=False, debug=False, num_devices=8)

    xc = nc.dram_tensor("xc", [T, N], F32, kind="ExternalInput")
    maskneg = nc.dram_tensor("maskneg", [N, N], BF16, kind="ExternalInput")
    wcomb0 = nc.dram_tensor("wcomb0", [HID, G4], F32, kind="ExternalInput")
    wih0 = nc.dram_tensor("wih0", [1, G4], F32, kind="ExternalInput")
    wcomb1 = nc.dram_tensor("wcomb1", [2 * HID, G4], F32, kind="ExternalInput")
    bias0 = nc.dram_tensor("bias0", [G4, 1], F32, kind="ExternalInput")
    bias1 = nc.dram_tensor("bias1", [G4, 1], F32, kind="ExternalInput")
    # layer-0 GAT weights: plain layout (input is contiguous 64-feat slab)
    waug0 = nc.dram_tensor("waug0", [HID, 66], F32, kind="ExternalInput")
    acomb0 = nc.dram_tensor("acomb0", [HID, 8], F32, kind="ExternalInput")
    # layer-1 GAT + head weights: padded layout (input rows 32*h+d, d<16)
    waug1p = nc.dram_tensor("waug1p", [128, 66], F32, kind="ExternalInput")
    acomb1p = nc.dram_tensor("acomb1p", [128, 8], F32, kind="ExternalInput")
    woutp = nc.dram_tensor("woutp", [128, 4], F32, kind="ExternalInput")
    wob = nc.dram_tensor("wob", [1, 4], F32, kind="ExternalInput")
    out_d = nc.dram_tensor("out", [OUT, N], F32, kind="ExternalOutput")

    ident2_d = nc.inline_tensor(np.eye(2, dtype=np.float32), name="ident2")
    ones66_np = np.zeros((1, 66), np.float32)
    ones66_np[0, 64] = 1.0
    ones66_d = nc.inline_tensor(ones66_np, name="ones66")

    with tile.TileContext(nc) as tc:
        with tc.tile_pool(name="perm", bufs=1) as perm:
            stg_pool = tc.tile_pool(name="stage", bufs=1)
            stg = stg_pool.__enter__()

            # ---- persistent tiles ----
            def load_w(dram, p, q, name, row0=None):
                f = stg.tile([p, q], F32, name=name + "_f", tag=name + "_f")
                if row0 is None:
                    nc.sync.dma_start(f[:], dram[:])
                else:
                    nc.sync.dma_start(f[:], dram[row0:row0 + p, :])
                r = perm.tile([p, q], F32R, name=name, tag=name)
                nc.vector.tensor_copy(r[:], f[:])
                return r

            wc0 = load_w(wcomb0, HID, G4, "wc0")
            wi0 = load_w(wih0, 1, G4, "wi0")
            wc1 = load_w(wcomb1, 2 * HID, G4, "wc1")
            wg0 = load_w(waug0, HID, 66, "wg0")
            wg1 = [load_w(waug1p, 64, 66, f"wg1_{i}", row0=64 * i)
                   for i in range(2)]
            ac0 = load_w(acomb0, HID, 8, "ac0")
            ac1 = [load_w(acomb1p, 64, 8, f"ac1_{i}", row0=64 * i)
                   for i in range(2)]
            wop = [load_w(woutp, 64, 4, f"wop{i}", row0=64 * i)
                   for i in range(2)]
            wo_one = load_w(wob, 1, 4, "wo_one")
            wg_one = load_w(ones66_d, 1, 66, "wg_one")

            bA, bB = [], []
            for li, bd in enumerate((bias0, bias1)):
                ba = perm.tile([128, 1], F32, name=f"bA{li}", tag=f"bA{li}")
                bb = perm.tile([128, 1], F32, name=f"bB{li}", tag=f"bB{li}")
                nc.sync.dma_start(ba[:], bd[0:128, :])
                nc.sync.dma_start(bb[:], bd[128:256, :])
                bA.append(ba)
                bB.append(bb)
            id2 = perm.tile([2, 2], F32)
            nc.sync.dma_start(id2[:], ident2_d[:])

            # ones row (f32r) for ones/bias matmul terms
            ones_f = stg.tile([1, N], F32)
            nc.gpsimd.memset(ones_f[:], 1.0)
            ones_r = perm.tile([1, N], F32R)
            nc.gpsimd.tensor_copy(ones_r[:], ones_f[:])

            # mask tiles (transposed adjacency, additive -300), resident
            mk = []
            for nt in range(NT):
                cnt = _ntcnt(nt)
                m = perm.tile([128, N], BF16, tag=f"mk{nt}", name=f"mk{nt}")
                nc.sync.dma_start(m[0:cnt, :], maskneg[nt * 128:nt * 128 + cnt, :])
                mk.append(m)

            # LSTM state [h0(0:64); h1(64:128)], f32r
            state1 = perm.tile([128, N], F32R)
            ctiles = [perm.tile([128, N], F32, tag=f"c{j}", name=f"c{j}")
                      for j in range(2)]

            # GAT outputs: per layer, two (64,N) tiles; head h at tile h//2,
            # rows 32*(h%2):32*(h%2)+16; other rows stay zero.
            zf = stg.tile([64, N], F32)
            nc.gpsimd.memset(zf[:], 0.0)
            gh = [[perm.tile([64, N], F32R, tag=f"gh{g}_{i}", name=f"gh{g}_{i}")
                   for i in range(2)] for g in range(2)]
            for g in range(2):
                for i in range(2):
                    nc.vector.tensor_copy(gh[g][i][:], zf[:])

            stg_pool.__exit__(None, None, None)

            rep_ctx = tc.For_i(0, reps, 1) if reps > 1 else None
            if rep_ctx is not None:
                rep_ctx.__enter__()

            # =================== LSTM ===================
            with tc.tile_pool(name="lwork", bufs=2) as lw, \
                 tc.tile_pool(name="lpsum", bufs=1, space="PSUM") as lp:
                for t in range(T):
                    xr_f = lw.tile([1, N], F32, tag="xrf", name="xrf")
                    nc.sync.dma_start(xr_f[:], xc[t:t + 1, :])
                    xr = lw.tile([1, N], F32R, tag="xrr", name="xrr")
                    nc.gpsimd.tensor_copy(xr[:], xr_f[:])
                    for l in range(2):
                        for ci, (s, w) in enumerate(CHUNKS):
                            gA = lp.tile([128, w], F32, tag=f"gA{ci}",
                                         name=f"gA{ci}")
                            gB = lp.tile([128, w], F32, tag=f"gB{ci}",
                                         name=f"gB{ci}")
                            if l == 0:
                                if t == 0:
                                    nc.tensor.matmul(gA[:], wi0[:, 0:128],
                                                     xr[:, s:s + w],
                                                     start=True, stop=True)
                                    nc.tensor.matmul(gB[:], wi0[:, 128:256],
                                                     xr[:, s:s + w],
                                                     start=True, stop=True)
                                else:
                                    nc.tensor.matmul(gA[:], wc0[:, 0:128],
                                                     state1[0:64, s:s + w],
                                                     start=True, stop=False)
                                    nc.tensor.matmul(gA[:], wi0[:, 0:128],
                                                     xr[:, s:s + w],
                                                     start=False, stop=True)
                                    nc.tensor.matmul(gB[:], wc0[:, 128:256],
                                                     state1[0:64, s:s + w],
                                                     start=True, stop=False)
                                    nc.tensor.matmul(gB[:], wi0[:, 128:256],
                                                     xr[:, s:s + w],
                                                     start=False, stop=True)
                            else:
                                if t == 0:
                                    nc.tensor.matmul(gA[:], wc1[0:64, 0:128],
                                                     state1[0:64, s:s + w],
                                                     start=True, stop=True)
                                    nc.tensor.matmul(gB[:], wc1[0:64, 128:256],
                                                     state1[0:64, s:s + w],
                                                     start=True, stop=True)
                                else:
                                    nc.tensor.matmul(gA[:], wc1[:, 0:128],
                                                     state1[:, s:s + w],
                                                     start=True, stop=True)
                                    nc.tensor.matmul(gB[:], wc1[:, 128:256],
                                                     state1[:, s:s + w],
                                                     start=True, stop=True)
                            # activations: gA=[i;f] sigmoid, gB=[g;o]
                            sA = lw.tile([128, w], F32, tag=f"sA{ci}",
                                         name=f"sA{ci}")
                            nc.scalar.activation(sA[:], gA[:], AF.Sigmoid,
                                                 bias=bA[l][:])
                            tg = lw.tile([64, w], F32, tag=f"tg{ci}",
                                         name=f"tg{ci}")
                            nc.scalar.activation(tg[:], gB[0:64, :], AF.Tanh,
                                                 bias=bB[l][0:64, :])
                            so = lw.tile([64, w], F32, tag=f"so{ci}",
                                         name=f"so{ci}")
                            nc.scalar.activation(so[:], gB[64:128, :], AF.Sigmoid,
                                                 bias=bB[l][64:128, :])
                            Cl = ctiles[l]
                            if t == 0:
                                nc.vector.tensor_mul(Cl[64:128, s:s + w],
                                                     sA[0:64, :], tg[:])
                            else:
                                M = lw.tile([64, 2 * w], F32, tag=f"M{ci}",
                                            name=f"M{ci}")
                                nc.vector.tensor_mul(M[:, 0:w], sA[0:64, :],
                                                     tg[:])
                                nc.gpsimd.tensor_mul(M[:, w:2 * w],
                                                     sA[64:128, :],
                                                     Cl[64:128, s:s + w])
                                nc.vector.tensor_add(Cl[64:128, s:s + w],
                                                     M[:, 0:w], M[:, w:2 * w])
                            tc_sb = lw.tile([64, w], F32, tag=f"tc{ci}",
                                            name=f"tc{ci}")
                            nc.scalar.activation(tc_sb[:], Cl[64:128, s:s + w],
                                                 AF.Tanh)
                            nc.vector.tensor_mul(
                                state1[64 * l:64 * l + 64, s:s + w],
                                so[:], tc_sb[:])

            # h1 (final top-layer hidden) to a base-0 feat-major tile
            h1t = perm.tile([64, N], F32R)
            nc.vector.tensor_copy(h1t[:], state1[64:128, :])

            # =================== GAT layers ===================
            for g in range(2):
                with tc.tile_pool(name=f"gw{g}", bufs=2) as gwp, \
                     tc.tile_pool(name=f"gp{g}", bufs=1, space="PSUM") as gp:
                    # hp_aug per node tile: (cnt, 66) cols 0:64 feats, 64=Z-ones
                    hpa = []
                    for nt in range(NT):
                        cnt = _ntcnt(nt)
                        ns = nt * 128
                        php = gp.tile([128, 66], F32, tag="php", name="php")
                        if g == 0:
                            nc.tensor.matmul(php[0:cnt, :],
                                             h1t[:, ns:ns + cnt],
                                             wg0[:],
                                             start=True, stop=False)
                        else:
                            for i in range(2):
                                nc.tensor.matmul(php[0:cnt, :],
                                                 gh[0][i][:, ns:ns + cnt],
                                                 wg1[i][:],
                                                 start=(i == 0), stop=False)
                        nc.tensor.matmul(php[0:cnt, :],
                                         ones_r[:, ns:ns + cnt],
                                         wg_one[:],
                                         start=False, stop=True)
                        hp = gwp.tile([128, 66], E_MM, tag=f"hp{nt}",
                                      name=f"hp{nt}", bufs=1)
                        nc.scalar.copy(hp[0:cnt, :], php[0:cnt, :])
                        hpa.append(hp)

                    for h in range(HEADS):
                        # e_src/e_dst rows for head h: (2, N) f32
                        esd = gwp.tile([2, N], F32, tag="esd", name="esd", bufs=1)
                        for ci, (s, w) in enumerate(CHUNKS):
                            pe = gp.tile([2, 512], F32, tag="pesd", name="pesd")
                            if g == 0:
                                nc.tensor.matmul(pe[:, 0:w],
                                                 ac0[:, 2 * h:2 * h + 2],
                                                 h1t[:, s:s + w],
                                                 start=True, stop=True)
                            else:
                                for i in range(2):
                                    nc.tensor.matmul(
                                        pe[:, 0:w],
                                        ac1[i][:, 2 * h:2 * h + 2],
                                        gh[0][i][:, s:s + w],
                                        start=(i == 0), stop=(i == 1))
                            nc.scalar.copy(esd[:, s:s + w], pe[:, 0:w])
                        # d columns via PE transpose: dcols[:, 2jt+1] = d
                        dcols = gwp.tile([128, 2 * NT], F32, tag="dcols",
                                         name="dcols", bufs=1)
                        for nt in range(NT):
                            cnt = _ntcnt(nt)
                            ptr = gp.tile([128, 2], F32, tag="pesd",
                                          name="ptr")
                            nc.tensor.transpose(ptr[0:cnt, :],
                                                esd[:, nt * 128:nt * 128 + cnt],
                                                id2[:])
                            nc.scalar.copy(dcols[0:cnt, 2 * nt:2 * nt + 2],
                                           ptr[0:cnt, :])
                        # s broadcast along partitions
                        if E_BF16:
                            esd_b = gwp.tile([1, N], BF16, tag="esdb",
                                             name="esdb")
                            nc.scalar.copy(esd_b[:], esd[0:1, :])
                            sb_src = esd_b[0:1, :]
                        else:
                            sb_src = esd[0:1, :]
                        sbc = gwp.tile([128, N], E_DT, tag="sbc", name="sbc", bufs=1)
                        nc.gpsimd.partition_broadcast(sbc[:], sb_src)

                        pho = gp.tile([16, N], F32, tag="pho", name="pho")
                        pz = gp.tile([2, N], F32, tag="pz", name="pz")
                        for nt in range(NT):
                            cnt = _ntcnt(nt)
                            tt = gwp.tile([128, N], E_DT, tag="tt", name="tt")
                            nc.vector.scalar_tensor_tensor(
                                tt[0:cnt, :], mk[nt][0:cnt, :],
                                dcols[0:cnt, 2 * nt + 1:2 * nt + 2],
                                sbc[0:cnt, :], ALU.add, ALU.add)
                            lr = gwp.tile([128, N], E_DT, tag="lr", name="lr")
                            nc.vector.scalar_tensor_tensor(
                                lr[0:cnt, :], tt[0:cnt, :], 0.2, tt[0:cnt, :],
                                ALU.mult, ALU.max)
                            em = gwp.tile([128, N], E_MM, tag="em", name="em")
                            nc.scalar.activation(em[0:cnt, :], lr[0:cnt, :],
                                                 AF.Exp)
                            for ci, (s, w) in enumerate(CHUNKS):
                                nc.tensor.matmul(
                                    pho[:, s:s + w],
                                    hpa[nt][0:cnt, 16 * h:16 * h + 16],
                                    em[0:cnt, s:s + w],
                                    start=(nt == 0), stop=(nt == NT - 1))
                                nc.tensor.matmul(
                                    pz[:, s:s + w],
                                    hpa[nt][0:cnt, 64:66],
                                    em[0:cnt, s:s + w],
                                    start=(nt == 0), stop=(nt == NT - 1))
                        # normalize + relu -> gh[g]
                        rz = gwp.tile([1, N], F32, tag="rz", name="rz", bufs=1)
                        nc.vector.reciprocal(rz[:], pz[0:1, :])
                        rzb = gwp.tile([16, N], F32, tag="rzb", name="rzb", bufs=1)
                        nc.gpsimd.partition_broadcast(rzb[:], rz[:])
                        rh = gwp.tile([16, N], F32, tag="rh", name="rh", bufs=1)
                        nc.scalar.activation(rh[:], pho[:], AF.Relu)
                        ro = 32 * (h % 2)
                        nc.vector.tensor_mul(gh[g][h // 2][ro:ro + 16, :],
                                             rh[:], rzb[:])

            # =================== output head ===================
            with tc.tile_pool(name="hw", bufs=1) as hw, \
                 tc.tile_pool(name="hp2", bufs=1, space="PSUM") as hp2:
                po = hp2.tile([4, N], F32, tag="po", name="po")
                for ci, (s, w) in enumerate(CHUNKS):
                    for i in range(2):
                        nc.tensor.matmul(po[:, s:s + w],
                                         wop[i][:],
                                         gh[1][i][:, s:s + w],
                                         start=(i == 0), stop=False)
                    nc.tensor.matmul(po[:, s:s + w], wo_one[:],
                                     ones_r[:, s:s + w],
                                     start=False, stop=True)
                osb = hw.tile([4, N], F32)
                nc.scalar.copy(osb[:], po[:])
                nc.sync.dma_start(out_d[:], osb[0:3, :])

            if rep_ctx is not None:
                rep_ctx.__exit__(None, None, None)

    nc.compile()
    return nc


_NC = None


def _pad64(a):
    """(64, q) rows c=16h+d -> (128, q) rows 32h+d (d<16), zeros elsewhere."""
    out = np.zeros((128, a.shape[1]), np.float32)
    for h in range(HEADS):
        out[32 * h:32 * h + 16] = a[16 * h:16 * h + 16]
    return out


def _pack(inputs):
    f32 = lambda a: np.asarray(a, dtype=np.float32)
    adj = np.asarray(inputs["adj"])
    maskneg = np.where(adj.T > 0, 0.0, MASKNEG).astype(ml_dtypes.bfloat16)
    W0, W1 = f32(inputs["gat0_W"]), f32(inputs["gat1_W"])

    def gat_w(W, asrc, adst):
        wa = np.zeros((HID, 66), np.float32)
        wa[:, 0:HID] = W.T
        acb = np.zeros((HID, 8), np.float32)
        for h in range(HEADS):
            blk = W[16 * h:16 * h + 16, :]  # (16, 64)
            acb[:, 2 * h] = f32(asrc)[h] @ blk
            acb[:, 2 * h + 1] = f32(adst)[h] @ blk
        return wa, acb

    wa0, acb0 = gat_w(W0, inputs["gat0_asrc"], inputs["gat0_adst"])
    wa1, acb1 = gat_w(W1, inputs["gat1_asrc"], inputs["gat1_adst"])
    wo = np.zeros((HID, 4), np.float32)
    wo[:, 0:OUT] = f32(inputs["out_W"]).T
    wob = np.zeros((1, 4), np.float32)
    wob[0, 0:OUT] = f32(inputs["out_b"])
    shared = {
        "maskneg": maskneg,
        "wcomb0": f32(inputs["lstm_Whh0"]).T.copy(),
        "wih0": f32(inputs["lstm_Wih0"]).T.copy(),
        "wcomb1": np.concatenate([f32(inputs["lstm_Wih1"]).T,
                                  f32(inputs["lstm_Whh1"]).T], 0),
        "bias0": (f32(inputs["lstm_bih0"]) +
                  f32(inputs["lstm_bhh0"])).reshape(G4, 1),
        "bias1": (f32(inputs["lstm_bih1"]) +
                  f32(inputs["lstm_bhh1"])).reshape(G4, 1),
        "waug0": wa0,
        "acomb0": acb0,
        "waug1p": _pad64(wa1),
        "acomb1p": _pad64(acb1),
        "woutp": _pad64(wo),
        "wob": wob,
    }
    x = f32(inputs["x"])
    return [dict(shared, xc=x[b].copy()) for b in range(B)]


def kernel(**inputs) -> np.ndarray:
    global _NC
    in_maps = _pack(inputs)
    if _NC is None:
        _NC = _build()
    res = run_bass_kernel_spmd(_NC, in_maps, list(range(B)))
    return np.stack([res.results[b]["out"] for b in range(B)], 0)
